# revision 24
# baseline (speedup 1.0000x reference)
"""Trainium2 Bass kernel for nn_Attention_27358941675773.

Reference computation (per batch b):
    q = x @ Q              [N, H]
    k = x @ K              [N, H]
    V = V_down @ V_up      [L, L]
    v = x @ V              [N, L]
    S = q @ k.T / 256      [N, N]
    out = softmax(S) @ v   [N, L]

Sharding: pure data-parallel over batch B=8 across the 8 NeuronCores
(one batch element per core); small params replicated. No collectives.

Per-core design (N=4096, L=256, H=128), evolved from the 183us baseline:
  - The Scalar engine's exp stream paces everything. Each ACTIVATE has a
    ~310-cycle pipeline-fill tax, so blocks 1-3 run their 32 key-tiles as
    10 PAIRS (one 2048-wide exp out of a 4-bank PSUM region) + 12
    singles, saving ~2.6us of scalar time per block. PSUM layout is
    exactly 8 banks: S2 [128,2048] + S1 [128,1024] + mid [128,1024].
  - Block 0 (which also runs all projections) keeps 1024-wide exps on a
    manual 3-view rotation over the same S2/S1 memory.
  - Input DMA: one DMA instruction sustains only ~70GB/s, so x ships as
    six DMAs spread across the gpsimd/sync/scalar queues, ordered so the
    pieces that gate the first projections land first.
  - w = x@V_down is projected in pair/quad batches (9 PSUM borrows, not
    32); qT halves f4-f7 are deferred into blocks 1/2 where they borrow
    S1 between singles.
  - Output drains (V_up matmul + 1/rowsum scale) run in the block-end
    singles window using S2 as scratch.
  - Softmax denominators: pairwise bf16 adds on DVE (lvl1/2 and, for
    blocks 0-2, lvl3-5 feeding a GpSimd partition-all-reduce off the
    critical path). Block 3 instead column-sums its tree partials on the
    PE with a ones vector into [1,1024] PSUM right at stream end, then
    DVE reciprocal + GpSimd partition-broadcast - ~7us faster than the
    all-reduce chain it replaces.
  - Output stored transposed [L, N] fp16; host un-transposes at gather.
"""

import os
import sys

import numpy as np

for _p in ("/opt/trn_rl_repo",):
    if _p not in sys.path and os.path.isdir(_p):
        sys.path.insert(0, _p)

B, N, L, H = 8, 4096, 256, 128
SCALER = 256.0
NB = 1024           # query-block (free dim of score tiles)
NBH = 512           # half block (one PSUM bank of fp32)
NT = N // NB        # 4 query blocks
MT = N // 128       # 32 key tiles of 128
P = 128


def _build():
    import concourse.bass as bass
    import concourse.tile as tile
    from concourse import bacc, bass_isa, mybir
    from contextlib import ExitStack

    f32 = mybir.dt.float32
    f16 = mybir.dt.float16
    bf16 = mybir.dt.bfloat16
    AF = mybir.ActivationFunctionType

    nc = bacc.Bacc(
        "TRN2", target_bir_lowering=False, debug=False, num_devices=B
    )

    xT_ext = nc.declare_dram_parameter("xT", [L, N], f16, isOutput=False)
    # Wq | Wk | Vd packed along the H axis: [L, 3H]
    wqkv_ext = nc.declare_dram_parameter("Wqkv", [L, 3 * H], f16, isOutput=False)
    vu_ext = nc.declare_dram_parameter("Vu", [H, L], f16, isOutput=False)
    # output stored transposed [L, N]; host un-transposes at gather
    out_ext = nc.declare_dram_parameter("out", [L, N], f16, isOutput=True)

    with tile.TileContext(nc) as tc, ExitStack() as ctx:
        persist = ctx.enter_context(tc.tile_pool(name="persist", bufs=1))

        # ---------------- phase A: input DMAs first ----------------
        xt16 = [
            persist.tile([P, N], f16, tag=f"xt{c}", name=f"xt16_{c}")
            for c in range(2)
        ]
        wqkv16 = persist.tile([P, 2 * 3 * H], f16)  # [l_chunk][l_in, 3h]
        vu16 = persist.tile([P, L], f16)            # V_up   [h, l]

        dum = persist.tile([1, 2], f32)
        nc.gpsimd.memset(dum[:], 0.0)
        nc.gpsimd.dma_start(xt16[0][:, 0:1024], xT_ext[0:P, 0:1024])
        nc.sync.dma_start(xt16[1][:, 0:1024], xT_ext[P:2 * P, 0:1024])
        for c in range(2):
            nc.scalar.dma_start(
                wqkv16[:, c * 3 * H:(c + 1) * 3 * H],
                wqkv_ext[c * P:(c + 1) * P, :],
            )
        wrm = persist.tile([P, NBH], bf16, name="wrm")
        nc.vector.memset(wrm[:], 0.0)
        nc.sync.dma_start(xt16[0][:, 1024:2048], xT_ext[0:P, 1024:2048])
        nc.sync.dma_start(xt16[1][:, 1024:2048], xT_ext[P:2 * P, 1024:2048])
        nc.gpsimd.dma_start(xt16[1][:, 2048:4096], xT_ext[P:2 * P, 2048:4096])
        nc.gpsimd.dma_start(xt16[0][:, 2048:4096], xT_ext[0:P, 2048:4096])
        nc.sync.dma_start(vu16[:], vu_ext[:, :])

        def qw_sl(c):
            return wqkv16[:, c * 3 * H + 0 * H: c * 3 * H + 1 * H]

        def kw_sl(c):
            return wqkv16[:, c * 3 * H + 1 * H: c * 3 * H + 2 * H]

        def vd_sl(c):
            return wqkv16[:, c * 3 * H + 2 * H: c * 3 * H + 3 * H]

        ones_bf = persist.tile([P, 1], bf16)
        nc.gpsimd.memset(ones_bf[:], 1.0)
        # touch Exp right away so the ~2.7us ACT table load overlaps the
        # input DMAs instead of delaying the first real exp
        nc.scalar.activation(dum[:, 1:2], dum[:, 0:1], AF.Exp)

        vu_bf = persist.tile([P, L], bf16)      # V_up as bf16 (out matmul)
        nc.vector.tensor_copy(vu_bf[:], vu16[:])
        qT16 = persist.tile([P, N], f16)        # q.T       [h, n]
        kT16 = persist.tile([P, N], f16)        # k.T       [h, m]
        w_sb = persist.tile([P, MT * H], bf16)  # x@V_down  [m_tile][m_in, h]

        # ------------- phases B+C: projections fused with attention -------
        with (
            tc.tile_pool(name="est1", bufs=24) as est1_pool,
            tc.tile_pool(name="est2", bufs=9) as est2_pool,
            tc.tile_pool(name="sb_small", bufs=4) as sb_small,
            tc.tile_pool(name="outfin", bufs=4) as outfin_pool,
            tc.tile_pool(name="psum2", bufs=1, space="PSUM") as psum2,
            tc.tile_pool(name="psum1", bufs=1, space="PSUM") as psum1,
            tc.tile_pool(name="mtp", bufs=1, space="PSUM") as mtp,
        ):
            # persistent PSUM regions, manually sliced (8 banks total)
            s2 = psum2.tile([P, 2 * NB], f32, tag="s2", name="s2")
            s1 = psum1.tile([P, NB], f32, tag="s1", name="s1")

            # block-0 three-view rotation over the same memory
            views = [s2[:, 0:NB], s2[:, NB:2 * NB], s1[:, 0:NB]]
            rot = {"i": 0}

            def next_ps():
                v = views[rot["i"] % 3]
                rot["i"] += 1
                return v

            est = {}      # (k, mt) -> bf16 [128, NB] exp score slices
            mtiles = {}   # k -> psum numerator mid^T [h, n] tile
            mscs = {}     # k -> numerator copy (bf16, SBUF)
            bc = {}       # k -> [128, NB] f32 broadcast 1/rowsum
            tree = {}     # (k, level, i) -> partial rowsum tiles

            def proj_qkT(wsl, dst, f, ps=None, cast_on_act=False):
                if ps is None:
                    ps = next_ps()
                for c in range(2):
                    nc.tensor.matmul(
                        ps[:, :NBH],
                        wsl(c),
                        xt16[c][:, f * NBH:(f + 1) * NBH],
                        start=(c == 0), stop=(c == 1),
                    )
                if cast_on_act:
                    nc.scalar.activation(
                        dst[:, f * NBH:(f + 1) * NBH], ps[:, :NBH], AF.Copy
                    )
                else:
                    nc.vector.tensor_copy(
                        dst[:, f * NBH:(f + 1) * NBH], ps[:, :NBH]
                    )

            def proj_wn(j0, n):
                # one rotation borrow + one cast for n w tiles
                ps = next_ps()
                for i in range(n):
                    mt = j0 + i
                    for c in range(2):
                        nc.tensor.matmul(
                            ps[:, i * H:(i + 1) * H],
                            xt16[c][:, mt * P:(mt + 1) * P],
                            vd_sl(c),
                            start=(c == 0), stop=(c == 1),
                        )
                nc.vector.tensor_copy(
                    w_sb[:, j0 * H:(j0 + n) * H], ps[:, :n * H]
                )

            def qk_exp0(k, mt):
                # block 0: single 1024-wide exp via the rotation
                ps = next_ps()
                for h in range(2):
                    nc.tensor.matmul(
                        ps[:, h * NBH:(h + 1) * NBH],
                        kT16[:, mt * P:(mt + 1) * P],
                        qT16[:, k * NB + h * NBH: k * NB + (h + 1) * NBH],
                        start=True, stop=True,
                    )
                e = est1_pool.tile([P, NB], bf16, tag="est1",
                                   name=f"est_{k}_{mt}")
                est[(k, mt)] = e
                nc.scalar.activation(e[:], ps[:], AF.Exp, scale=1.0 / SCALER)

            def qk_exp_pair(k, mt):
                # key tiles mt, mt+1 -> one 2048-wide exp out of S2
                for i in range(2):
                    for h in range(2):
                        nc.tensor.matmul(
                            s2[:, i * NB + h * NBH: i * NB + (h + 1) * NBH],
                            kT16[:, (mt + i) * P:(mt + i + 1) * P],
                            qT16[:, k * NB + h * NBH: k * NB + (h + 1) * NBH],
                            start=True, stop=True,
                        )
                e2 = est2_pool.tile([P, 2 * NB], bf16, tag="est2",
                                    name=f"est2_{k}_{mt}")
                est[(k, mt)] = e2[:, 0:NB]
                est[(k, mt + 1)] = e2[:, NB:2 * NB]
                nc.scalar.activation(e2[:], s2[:], AF.Exp, scale=1.0 / SCALER)

            def qk_exp_single(k, mt):
                for h in range(2):
                    nc.tensor.matmul(
                        s1[:, h * NBH:(h + 1) * NBH],
                        kT16[:, mt * P:(mt + 1) * P],
                        qT16[:, k * NB + h * NBH: k * NB + (h + 1) * NBH],
                        start=True, stop=True,
                    )
                e = est1_pool.tile([P, NB], bf16, tag="est1",
                                   name=f"est_{k}_{mt}")
                est[(k, mt)] = e
                nc.scalar.activation(e[:], s1[:], AF.Exp, scale=1.0 / SCALER)

            def tree_adds(k, mt):
                # lazily build the pairwise rowsum tree on DVE (bf16; the
                # all-reduce upcasts internally and the denominator only
                # needs ~0.4%)
                if mt % 2 == 1:
                    t = sb_small.tile([P, NB], bf16, tag="t1", bufs=3,
                                      name=f"t1_{k}_{mt}")
                    nc.vector.tensor_add(t[:], est[(k, mt - 1)][:], est[(k, mt)][:])
                    tree[(k, 1, mt // 2)] = t
                if mt % 4 == 3:
                    t = sb_small.tile([P, NB], bf16, tag="t2", bufs=3,
                                      name=f"t2_{k}_{mt}")
                    nc.vector.tensor_add(
                        t[:], tree[(k, 1, mt // 2 - 1)][:], tree[(k, 1, mt // 2)][:]
                    )
                    tree[(k, 2, mt // 4)] = t
                if mt % 8 == 7:
                    t = sb_small.tile([P, NB], bf16, tag="t3", bufs=4,
                                      name=f"t3_{k}_{mt}")
                    nc.vector.tensor_add(
                        t[:], tree[(k, 2, mt // 4 - 1)][:], tree[(k, 2, mt // 4)][:]
                    )
                    tree[(k, 3, mt // 8)] = t
                if k < NT - 1:
                    if mt % 16 == 15:
                        t = sb_small.tile([P, NB], bf16, tag="t4", bufs=2,
                                          name=f"t4_{k}_{mt}")
                        nc.vector.tensor_add(
                            t[:], tree[(k, 3, mt // 8 - 1)][:], tree[(k, 3, mt // 8)][:]
                        )
                        tree[(k, 4, mt // 16)] = t
                    if mt == 31:
                        t = sb_small.tile([P, NB], bf16, tag="t5", bufs=2,
                                          name=f"t5_{k}")
                        nc.vector.tensor_add(
                            t[:], tree[(k, 4, 0)][:], tree[(k, 4, 1)][:]
                        )
                        tree[(k, 5, 0)] = t

            def rowsum_finish(k):
                # all-reduce over partitions on GpSimd (off critical path
                # for blocks 0-2), then fast reciprocal on DVE
                rsb = sb_small.tile([P, NB], f32, tag="rsb", bufs=2,
                                    name=f"rsb_{k}")
                nc.gpsimd.partition_all_reduce(
                    rsb[:], tree[(k, 5, 0)][:], channels=P,
                    reduce_op=bass_isa.ReduceOp.add,
                )
                bc[k] = rsb

            def recip_bc(k):
                bck = sb_small.tile([P, NB], f32, tag="bc", bufs=2,
                                    name=f"bc_{k}")
                nc.vector.reciprocal_approx_fast(bck[:], bc[k][:])
                bc[k] = bck

            def norm_mid(k):
                # copy mid out of PSUM (frees the mtp slot for the next
                # block); on DVE - the scalar stream is the pacer
                msc = sb_small.tile([P, NB], bf16, tag="msc", bufs=2,
                                    name=f"msc_{k}")
                nc.vector.tensor_copy(msc[:], mtiles[k][:])
                mscs[k] = msc

            def drain_out(k):
                # apply V_up, normalize by 1/rowsum, store transposed.
                # Runs in the block-end singles window with S2 as scratch:
                # the pair slot is idle from exp(P9) until the next block's
                # first pair QK.
                for lt in range(2):
                    op_v = s2[:, lt * NB:(lt + 1) * NB]
                    for h in range(2):
                        nc.tensor.matmul(
                            op_v[:, h * NBH:(h + 1) * NBH],
                            vu_bf[:, lt * P:(lt + 1) * P],
                            mscs[k][:, h * NBH:(h + 1) * NBH],
                            start=True, stop=True,
                        )
                    fin = outfin_pool.tile([P, NB], f16, tag="fin")
                    nc.vector.tensor_mul(fin[:], op_v[:], bc[k][:])
                    nc.sync.dma_start(
                        out_ext[lt * P:(lt + 1) * P, k * NB:(k + 1) * NB],
                        fin[:],
                    )

            def pv2(kk, j, mid):
                for h in range(2):
                    nc.tensor.matmul(
                        mid[:, h * NBH:(h + 1) * NBH],
                        w_sb[:, j * H:(j + 1) * H],
                        est[(kk, j)][:, h * NBH:(h + 1) * NBH],
                        start=(j == 0), stop=(j == MT - 1),
                    )

            # PE warm-up: junk matmuls while the x DMA is in flight
            for i in range(6):
                ps = next_ps()
                nc.tensor.matmul(
                    ps[:, :NBH], wrm[:, :P], wrm[:], start=True, stop=True
                )

            # head: the first QK tiles need qT/kT half-blocks 0,1
            proj_qkT(qw_sl, qT16, 0, cast_on_act=True)
            proj_qkT(qw_sl, qT16, 1)
            proj_qkT(kw_sl, kT16, 0, cast_on_act=True)
            proj_qkT(kw_sl, kT16, 1)

            for k in range(NT):
                for mt in range(MT):
                    if k == 0:
                        qk_exp0(k, mt)
                    elif mt % 3 == 0 and mt < 30:
                        qk_exp_pair(k, mt)
                    elif mt % 3 == 2 or mt >= 30:
                        qk_exp_single(k, mt)
                    if k == 0:
                        if mt in (0, 2):
                            proj_wn(mt, 2)
                        elif mt in (4, 6, 8, 10, 12, 14):
                            proj_wn(4 + (mt - 4) * 2, 4)
                        elif mt == 16:
                            proj_wn(28, 4)
                        if mt in (1, 3, 5, 7, 13, 15):
                            proj_qkT(
                                kw_sl, kT16,
                                {1: 2, 3: 3, 5: 4, 7: 5, 13: 6, 15: 7}[mt],
                            )
                        elif mt in (9, 11):
                            proj_qkT(qw_sl, qT16, 2 + (mt - 9) // 2)
                    if k in (1, 2) and mt in (5, 14):
                        # deferred qT halves borrow S1 right after a
                        # single's exp; the next single QK waits the cast
                        proj_qkT(
                            qw_sl, qT16, 4 + 2 * (k - 1) + (1 if mt == 14 else 0),
                            ps=s1[:, 0:NB],
                        )
                    if k >= 1 and mt <= 15:
                        pv2(k - 1, 16 + mt, mtiles[k - 1])
                    if mt == 16:
                        mid = mtp.tile([P, NB], f32, tag="mtp", name=f"mid_{k}")
                        mtiles[k] = mid
                    if mt >= 16:
                        pv2(k, mt - 16, mtiles[k])
                    if k == NT - 1 and mt >= 18 and mt % 2 == 0:
                        # last block: pull part of the epilogue forward (j
                        # lags mt by >=2 so the est slice always exists)
                        pv2(k, 16 + (mt - 18) // 2, mtiles[k])
                    if k >= 1 and mt == 20:
                        recip_bc(k - 1)
                    if k >= 1 and mt == 15:
                        norm_mid(k - 1)
                    tree_adds(k, mt)
                    if k >= 1 and mt == 29:
                        drain_out(k - 1)
                if k < NT - 1:
                    rowsum_finish(k)

            # ---------------- epilogue: finish block 3 and drain ----------
            k3 = NT - 1
            # PV halves interleaved per j so both halves of mid stop early
            for j in range(23, MT):
                pv2(k3, j, mtiles[k3])
            mscs_h = []
            for h in range(2):
                msc_h = sb_small.tile([P, NBH], bf16, tag="msch", bufs=2,
                                      name=f"msch_{h}")
                nc.scalar.activation(
                    msc_h[:], mtiles[k3][:, h * NBH:(h + 1) * NBH], AF.Copy
                )
                mscs_h.append(msc_h)
            # block-3 rowsum: PE colsum with a ones vector into [1, NB]
            # PSUM (S1 scratch) over the finest tree tiles still alive;
            # skips tree levels 4/5 and the 6.7us all-reduce
            cs = s1[0:1, 0:NB]
            csrc = [tree[(k3, 3, 0)], tree[(k3, 3, 1)], tree[(k3, 3, 2)],
                    tree[(k3, 2, 6)], tree[(k3, 1, 14)], tree[(k3, 1, 15)]]
            for i, t in enumerate(csrc):
                for hh in range(2):
                    nc.tensor.matmul(
                        cs[0:1, hh * NBH:(hh + 1) * NBH],
                        ones_bf[:, 0:1],
                        t[:, hh * NBH:(hh + 1) * NBH],
                        start=(i == 0), stop=(i == len(csrc) - 1),
                    )
            # denominator chain: DVE reciprocal straight out of PSUM, then
            # GpSimd partition-broadcast in halves
            rcp3 = sb_small.tile([1, NB], f32, tag="rcp3", bufs=1)
            nc.vector.reciprocal_approx_fast(rcp3[0:1, :], cs[0:1, :])
            bc3 = sb_small.tile([P, NB], f32, tag="bc3", bufs=1)
            for h in range(2):
                nc.gpsimd.partition_broadcast(
                    bc3[:, h * NBH:(h + 1) * NBH],
                    rcp3[0:1, h * NBH:(h + 1) * NBH], channels=P,
                )

            fins = [
                outfin_pool.tile([P, NB], f16, tag="fine", bufs=2,
                                 name=f"fine_{lt}")
                for lt in range(2)
            ]
            ops = {}
            for h in range(2):
                op_v = s2[:, h * NB:(h + 1) * NB]
                for lt in range(2):
                    nc.tensor.matmul(
                        op_v[:, lt * NBH:(lt + 1) * NBH],
                        vu_bf[:, lt * P:(lt + 1) * P], mscs_h[h][:],
                        start=True, stop=True,
                    )
                ops[h] = op_v
            for h in range(2):
                for lt in range(2):
                    nc.vector.tensor_mul(
                        fins[lt][:, h * NBH:(h + 1) * NBH],
                        ops[h][:, lt * NBH:(lt + 1) * NBH],
                        bc3[:, h * NBH:(h + 1) * NBH],
                    )
                    if h == 1:
                        nc.sync.dma_start(
                            out_ext[lt * P:(lt + 1) * P, k3 * NB:(k3 + 1) * NB],
                            fins[lt][:],
                        )

    if not nc.is_finalized():
        nc.finalize()
    return nc


_GRAPH_CACHE = {}


def _get_graph():
    if "nc" not in _GRAPH_CACHE:
        _GRAPH_CACHE["nc"] = _build()
    return _GRAPH_CACHE["nc"]


def run(inputs: dict, trace: bool = False):
    """Run the SPMD kernel on 8 cores. Returns (output, BassKernelResults)."""
    from concourse.bass_utils import run_bass_kernel_spmd

    x = np.asarray(inputs["x"], dtype=np.float32)
    Q = np.asarray(inputs["Q"], dtype=np.float32)[0]
    K = np.asarray(inputs["K"], dtype=np.float32)[0]
    Vd = np.asarray(inputs["V_down"], dtype=np.float32)[0]
    Vu = np.asarray(inputs["V_up"], dtype=np.float32)[0]

    wqkv = np.ascontiguousarray(
        np.concatenate([Q, K, Vd], axis=1)
    ).astype(np.float16)
    vu = np.ascontiguousarray(Vu).astype(np.float16)

    in_maps = []
    for b in range(B):
        in_maps.append({
            "xT": np.ascontiguousarray(x[b].T).astype(np.float16),
            "Wqkv": wqkv,
            "Vu": vu,
        })

    nc = _get_graph()
    res = run_bass_kernel_spmd(nc, in_maps, core_ids=list(range(B)), trace=trace)
    # device output is [L, N] per core; un-transpose during the gather
    out = np.stack([np.asarray(res.results[i]["out"]).astype(np.float32).T for i in range(B)])
    return np.ascontiguousarray(out, dtype=np.float32), res


def kernel(**inputs) -> np.ndarray:
    out, _ = run(inputs, trace=False)
    return out


# revision 25
# speedup vs baseline: 1.2365x; 1.2365x over previous
"""Trainium2 Bass kernel for nn_Attention_27358941675773.

Reference computation (per batch b):
    q = x @ Q              [N, H]
    k = x @ K              [N, H]
    V = V_down @ V_up      [L, L]
    v = x @ V              [N, L]
    S = q @ k.T / 256      [N, N]
    out = softmax(S) @ v   [N, L]

Sharding: pure data-parallel over batch B=8 across the 8 NeuronCores
(one batch element per core); small params replicated. No collectives.

Per-core kernel design (N=4096, L=256, H=128): see baseline docstring in
kernel_baseline.py. Round-1 changes vs the 183us baseline:
  - Input DMA: 7 large descriptors (x as 4x [128,2048], packed Wqkv as
    2x [128,384], Vu) issued before anything else on the Sync queue so x
    lands ~10us earlier; kills the block-0 exp bubbles that came from
    projection matmuls head-of-line blocking on late x chunks.
  - Head: 6 warm-up matmuls (was 16), projections start as soon as the
    first x chunk lands; exp stream starts ~3us earlier.
  - Tail: block 3's softmax denominator no longer goes through the
    ~6.7us GpSimd PartitionAllReduce + full f32 tree. Instead the lvl-3
    rowsum partials (bf16) are column-summed on the PE with a ones
    vector into a [1,1024] PSUM tile (8 small matmuls, 6 of them
    pre-run), reciprocal'd on DVE, and partition-broadcast on GpSimd.
  - Output DMA triggers moved from GpSimd (SWDGE, 0.64us each) to the
    otherwise-idle Sync engine; epilogue fins merged into 2 DMAs.
"""

import os
import sys

import numpy as np

for _p in ("/opt/trn_rl_repo",):
    if _p not in sys.path and os.path.isdir(_p):
        sys.path.insert(0, _p)

B, N, L, H = 8, 4096, 256, 128
SCALER = 256.0
NB = 1024           # query-block (free dim of score tiles)
NBH = 512           # half block (one PSUM bank of fp32)
NT = N // NB        # 4 query blocks
MT = N // 128       # 32 key tiles of 128
P = 128


def _build():
    import concourse.bass as bass
    import concourse.tile as tile
    from concourse import bacc, bass_isa, mybir
    from contextlib import ExitStack

    f32 = mybir.dt.float32
    f16 = mybir.dt.float16
    bf16 = mybir.dt.bfloat16
    AF = mybir.ActivationFunctionType

    nc = bacc.Bacc(
        "TRN2", target_bir_lowering=False, debug=False, num_devices=B
    )

    xT_ext = nc.declare_dram_parameter("xT", [L, N], f16, isOutput=False)
    # Wq | Wk | Vd packed along the H axis: [L, 3H]
    wqkv_ext = nc.declare_dram_parameter("Wqkv", [L, 3 * H], f16, isOutput=False)
    vu_ext = nc.declare_dram_parameter("Vu", [H, L], f16, isOutput=False)
    # output stored transposed [L, N]; host un-transposes at gather
    out_ext = nc.declare_dram_parameter("out", [L, N], f16, isOutput=True)

    with tile.TileContext(nc) as tc, ExitStack() as ctx:
        persist = ctx.enter_context(tc.tile_pool(name="persist", bufs=1))

        # ---------------- phase A: input DMAs first ----------------
        # x lands in two [128, 4096] tiles (one per l-chunk). A single DMA
        # instruction sustains only ~70 GB/s, so the loads are spread
        # across four engine queues (gpsimd SWDGE + sync + scalar +
        # vector HWDGE) to run in parallel; the s0 pieces that gate the
        # first qT/kT projections go first on the two fastest-clearing
        # queues.
        xt16 = [
            persist.tile([P, N], f16, tag=f"xt{c}", name=f"xt16_{c}")
            for c in range(2)
        ]
        wqkv16 = persist.tile([P, 2 * 3 * H], f16)  # [l_chunk][l_in, 3h]
        vu16 = persist.tile([P, L], f16)            # V_up   [h, l]

        dum = persist.tile([1, 2], f32)
        nc.gpsimd.memset(dum[:], 0.0)
        nc.gpsimd.dma_start(xt16[0][:, 0:1024], xT_ext[0:P, 0:1024])
        nc.sync.dma_start(xt16[1][:, 0:1024], xT_ext[P:2 * P, 0:1024])
        for c in range(2):
            nc.scalar.dma_start(
                wqkv16[:, c * 3 * H:(c + 1) * 3 * H],
                wqkv_ext[c * P:(c + 1) * P, :],
            )
        wrm = persist.tile([P, NBH], bf16, name="wrm")
        nc.vector.memset(wrm[:], 0.0)
        nc.sync.dma_start(xt16[0][:, 1024:2048], xT_ext[0:P, 1024:2048])
        nc.sync.dma_start(xt16[1][:, 1024:2048], xT_ext[P:2 * P, 1024:2048])
        nc.gpsimd.dma_start(xt16[1][:, 2048:4096], xT_ext[P:2 * P, 2048:4096])
        nc.gpsimd.dma_start(xt16[0][:, 2048:4096], xT_ext[0:P, 2048:4096])
        nc.sync.dma_start(vu16[:], vu_ext[:, :])

        def qw_sl(c):
            return wqkv16[:, c * 3 * H + 0 * H: c * 3 * H + 1 * H]

        def kw_sl(c):
            return wqkv16[:, c * 3 * H + 1 * H: c * 3 * H + 2 * H]

        def vd_sl(c):
            return wqkv16[:, c * 3 * H + 2 * H: c * 3 * H + 3 * H]

        ones_bf = persist.tile([P, 1], bf16)
        nc.gpsimd.memset(ones_bf[:], 1.0)
        # touch Exp right away so the ~2.7us ACT table load overlaps the
        # input DMAs instead of delaying the first real exp
        nc.scalar.activation(dum[:, 1:2], dum[:, 0:1], AF.Exp)

        vu_bf = persist.tile([P, L], bf16)      # V_up as bf16 (out matmul)
        nc.vector.tensor_copy(vu_bf[:], vu16[:])
        qT16 = persist.tile([P, N], f16)        # q.T       [h, n]
        kT16 = persist.tile([P, N], f16)        # k.T       [h, m]
        w_sb = persist.tile([P, MT * H], bf16)  # x@V_down  [m_tile][m_in, h]

        # ------------- phases B+C: projections fused with attention -------
        with (
            tc.tile_pool(name="est", bufs=40) as est_pool,
            tc.tile_pool(name="sb_small", bufs=4) as sb_small,
            tc.tile_pool(name="outfin", bufs=4) as outfin_pool,
            tc.tile_pool(name="stp", bufs=3, space="PSUM") as stp,
            tc.tile_pool(name="mtp", bufs=1, space="PSUM") as mtp,
        ):
            est = {}      # (k, mt) -> bf16 [128, NB] exp score tiles
            mtiles = {}   # k -> psum numerator mid^T [h, n] tile
            mscs = {}     # k -> normalized mid (f16, SBUF)
            bc = {}       # k -> [128, NB] f32 broadcast 1/rowsum
            tree = {}     # (k, level, i) -> partial rowsum tiles

            def proj_qkT(wsl, dst, f, cast_on_act=False):
                ps = stp.tile([P, NB], f32, tag="stp", name=f"pjp_{f}")
                for c in range(2):
                    nc.tensor.matmul(
                        ps[:, :NBH],
                        wsl(c),
                        xt16[c][:, f * NBH:(f + 1) * NBH],
                        start=(c == 0), stop=(c == 1),
                    )
                if cast_on_act:
                    nc.scalar.activation(
                        dst[:, f * NBH:(f + 1) * NBH], ps[:, :NBH], AF.Copy
                    )
                else:
                    nc.vector.tensor_copy(
                        dst[:, f * NBH:(f + 1) * NBH], ps[:, :NBH]
                    )

            def proj_wn(j0, n):
                # one ring borrow + one cast for n w tiles (j=j0..j0+n-1)
                ps = stp.tile([P, NB], f32, tag="stp", name=f"pjw_{j0}")
                for i in range(n):
                    mt = j0 + i
                    for c in range(2):
                        nc.tensor.matmul(
                            ps[:, i * H:(i + 1) * H],
                            xt16[c][:, mt * P:(mt + 1) * P],
                            vd_sl(c),
                            start=(c == 0), stop=(c == 1),
                        )
                nc.vector.tensor_copy(
                    w_sb[:, j0 * H:(j0 + n) * H], ps[:, :n * H]
                )

            def qk_exp(k, mt):
                ps = stp.tile([P, NB], f32, tag="stp", name=f"qk_{k}_{mt}")
                for h in range(2):
                    nc.tensor.matmul(
                        ps[:, h * NBH:(h + 1) * NBH],
                        kT16[:, mt * P:(mt + 1) * P],
                        qT16[:, k * NB + h * NBH: k * NB + (h + 1) * NBH],
                        start=True, stop=True,
                    )
                e = est_pool.tile([P, NB], bf16, tag="est", name=f"est_{k}_{mt}")
                est[(k, mt)] = e
                nc.scalar.activation(e[:], ps[:], AF.Exp, scale=1.0 / SCALER)

            def tree_adds(k, mt):
                # lazily build the pairwise rowsum tree on DVE; for blocks
                # 0-2 levels 4/5 accumulate in fp32 ahead of the GpSimd
                # all-reduce; block 3 stops at bf16 level 3 (the PE colsum
                # consumes those tiles directly at the tail).
                if mt % 2 == 1:
                    t = sb_small.tile([P, NB], bf16, tag="t1", bufs=3,
                                      name=f"t1_{k}_{mt}")
                    nc.vector.tensor_add(t[:], est[(k, mt - 1)][:], est[(k, mt)][:])
                    tree[(k, 1, mt // 2)] = t
                if mt % 4 == 3:
                    t = sb_small.tile([P, NB], bf16, tag="t2", bufs=3,
                                      name=f"t2_{k}_{mt}")
                    nc.vector.tensor_add(
                        t[:], tree[(k, 1, mt // 2 - 1)][:], tree[(k, 1, mt // 2)][:]
                    )
                    tree[(k, 2, mt // 4)] = t
                if mt % 8 == 7:
                    t = sb_small.tile([P, NB], bf16, tag="t3", bufs=4,
                                      name=f"t3_{k}_{mt}")
                    nc.vector.tensor_add(
                        t[:], tree[(k, 2, mt // 4 - 1)][:], tree[(k, 2, mt // 4)][:]
                    )
                    tree[(k, 3, mt // 8)] = t
                if k < NT - 1:
                    # bf16 is fine here: the partition all-reduce upcasts to
                    # f32 internally and the denominator only needs ~0.4%.
                    if mt % 16 == 15:
                        t = sb_small.tile([P, NB], bf16, tag="t4", bufs=2,
                                          name=f"t4_{k}_{mt}")
                        nc.vector.tensor_add(
                            t[:], tree[(k, 3, mt // 8 - 1)][:], tree[(k, 3, mt // 8)][:]
                        )
                        tree[(k, 4, mt // 16)] = t
                    if mt == 31:
                        t = sb_small.tile([P, NB], bf16, tag="t5", bufs=2,
                                          name=f"t5_{k}")
                        nc.vector.tensor_add(
                            t[:], tree[(k, 4, 0)][:], tree[(k, 4, 1)][:]
                        )
                        tree[(k, 5, 0)] = t

            def rowsum_finish(k):
                # all-reduce over partitions on GpSimd (systolic daisy chain,
                # broadcast result), then fast reciprocal on DVE
                rsb = sb_small.tile([P, NB], f32, tag="rsb", bufs=2,
                                    name=f"rsb_{k}")
                nc.gpsimd.partition_all_reduce(
                    rsb[:], tree[(k, 5, 0)][:], channels=P,
                    reduce_op=bass_isa.ReduceOp.add,
                )
                bc[k] = rsb

            def recip_bc(k):
                # deferred so the Vector FIFO never head-of-line blocks on
                # the GpSimd PartitionAllReduce (finished a while ago)
                bck = sb_small.tile([P, NB], f32, tag="bc", bufs=2,
                                    name=f"bc_{k}")
                nc.vector.reciprocal_approx_fast(bck[:], bc[k][:])
                bc[k] = bck

            def norm_mid(k):
                # plain copy on the Scalar engine: it rides the exp stream,
                # so the mid PSUM tile frees right on schedule and the next
                # block's first PV matmul never stalls. The 1/rowsum scale
                # moves to the fin stage (linear, commutes with V_up).
                msc = sb_small.tile([P, NB], bf16, tag="msc", bufs=2,
                                    name=f"msc_{k}")
                nc.scalar.activation(msc[:], mtiles[k][:], AF.Copy)
                mscs[k] = msc

            def drain_out(k, lt):
                # apply V_up, normalize by 1/rowsum, store transposed (f16).
                # The two row-halves drain at different mts (20 and 24) so
                # only one stp ring slot is borrowed at a time and the
                # Vector queue sees one fin mul per call, not two.
                op = stp.tile([P, NB], f32, tag="stp", name=f"op_{k}_{lt}")
                for h in range(2):
                    nc.tensor.matmul(
                        op[:, h * NBH:(h + 1) * NBH],
                        vu_bf[:, lt * P:(lt + 1) * P],
                        mscs[k][:, h * NBH:(h + 1) * NBH],
                        start=True, stop=True,
                    )
                fin = outfin_pool.tile([P, NB], f16, tag="fin")
                nc.vector.tensor_mul(fin[:], op[:], bc[k][:])
                nc.sync.dma_start(
                    out_ext[lt * P:(lt + 1) * P, k * NB:(k + 1) * NB],
                    fin[:],
                )

            def pv2(kk, j, mid):
                for h in range(2):
                    nc.tensor.matmul(
                        mid[:, h * NBH:(h + 1) * NBH],
                        w_sb[:, j * H:(j + 1) * H],
                        est[(kk, j)][:, h * NBH:(h + 1) * NBH],
                        start=(j == 0), stop=(j == MT - 1),
                    )

            # PE warm-up: junk matmuls while the x DMA is in flight, so the
            # HAM clock gate is already ramping when real work starts
            for i in range(6):
                ps = stp.tile([P, NB], f32, tag="stp", name=f"warm_{i}")
                nc.tensor.matmul(
                    ps[:, :NBH], wrm[:, :P], wrm[:], start=True, stop=True
                )

            # Uniform half-block-lagged schedule: during block k, PE runs
            # QK(k) plus the oldest pending attention@w work (last half of
            # block k-1, then first half of block k), so per-mt PE load is a
            # constant 4 matmuls and the Scalar engine's exp stream paces
            # everything. Block 0 uses the projection matmuls as its filler.
            # head: the first QK tiles need qT/kT half-blocks 0,1 (chunk s0)
            proj_qkT(qw_sl, qT16, 0, cast_on_act=True)
            proj_qkT(qw_sl, qT16, 1)
            proj_qkT(kw_sl, kT16, 0, cast_on_act=True)
            proj_qkT(kw_sl, kT16, 1)

            for k in range(NT):
                for mt in range(MT):
                    qk_exp(k, mt)
                    if k == 0:
                        # w projections batched (pairs while only the s0 x
                        # chunk has landed, quads after) - 9 ring borrows
                        # instead of 32; kT/qT half-block projections on the
                        # odd mts. qT f4-f7 are deferred to blocks 1/2 (they
                        # are only needed at the starts of blocks 2/3).
                        if mt in (0, 2):
                            proj_wn(mt, 2)
                        elif mt in (4, 6, 8, 10, 12, 14):
                            proj_wn(4 + (mt - 4) * 2, 4)
                        elif mt == 16:
                            proj_wn(28, 4)
                        if mt in (1, 3, 5, 7, 13, 15):
                            proj_qkT(
                                kw_sl, kT16,
                                {1: 2, 3: 3, 5: 4, 7: 5, 13: 6, 15: 7}[mt],
                            )
                        elif mt in (9, 11):
                            proj_qkT(qw_sl, qT16, 2 + (mt - 9) // 2)
                    if k in (1, 2) and mt in (1, 3):
                        proj_qkT(qw_sl, qT16, 4 + 2 * (k - 1) + (mt - 1) // 2)
                    if k >= 1 and mt <= 15:
                        pv2(k - 1, 16 + mt, mtiles[k - 1])
                    if mt == 16:
                        mid = mtp.tile([P, NB], f32, tag="mtp", name=f"mid_{k}")
                        mtiles[k] = mid
                    if mt >= 16:
                        pv2(k, mt - 16, mtiles[k])
                    if k == NT - 1 and mt >= 18 and mt % 2 == 0:
                        # last block: pull forward part of the epilogue,
                        # spread at one extra pv2 every other mt so the PE
                        # never exceeds the exp cadence (j lags mt by >=2 so
                        # the est tile always exists)
                        pv2(k, 16 + (mt - 18) // 2, mtiles[k])
                    if k >= 1 and mt == 10:
                        recip_bc(k - 1)
                    if k >= 1 and mt == 15:
                        norm_mid(k - 1)
                    tree_adds(k, mt)
                    if k >= 1 and mt in (20, 24):
                        drain_out(k - 1, (mt - 20) // 4)
                    if k == NT - 1 and mt == 31:
                        # block-3 rowsum: PE colsum with a ones vector into
                        # [1, NB] PSUM over the finest set of tree tiles
                        # still alive in their rings: lvl3 0-2 (keys 0-3071,
                        # done mid-block) + lvl2_6 (keys 3072-3583) + the
                        # last two lvl1 pairs. The tail chain then skips
                        # tree levels 2/3 after the final exp.
                        cs = stp.tile([1, NB], f32, tag="stp", name="cs3")
                        bc[NT - 1] = cs  # placeholder; replaced below
                        pre = [tree[(k, 3, 0)], tree[(k, 3, 1)], tree[(k, 3, 2)]]
                        for i, t in enumerate(pre):
                            for h in range(2):
                                nc.tensor.matmul(
                                    cs[0:1, h * NBH:(h + 1) * NBH],
                                    ones_bf[:, 0:1],
                                    t[:, h * NBH:(h + 1) * NBH],
                                    start=(i == 0), stop=False,
                                )
                if k < NT - 1:
                    rowsum_finish(k)

            # epilogue: finish block 3's product and drain it
            k3 = NT - 1
            cs = bc[k3]
            # PV halves interleaved per j so both halves of mid stop ~2us
            # earlier than the half-serialized order
            for j in range(23, MT):
                pv2(k3, j, mtiles[k3])
            # last colsum half-matmuls: lvl2_6 (keys 3072-3583) and the two
            # final lvl1 pairs - the tail chain skips tree levels 2/3
            for t in (tree[(k3, 2, 6)], tree[(k3, 1, 14)], tree[(k3, 1, 15)]):
                for hh in range(2):
                    nc.tensor.matmul(
                        cs[0:1, hh * NBH:(hh + 1) * NBH],
                        ones_bf[:, 0:1],
                        t[:, hh * NBH:(hh + 1) * NBH],
                        start=False, stop=(t is tree[(k3, 1, 15)]),
                    )
            mscs_h = []
            for h in range(2):
                msc_h = sb_small.tile([P, NBH], bf16, tag="msch", bufs=2,
                                      name=f"msch_{h}")
                nc.scalar.activation(
                    msc_h[:], mtiles[k3][:, h * NBH:(h + 1) * NBH], AF.Copy
                )
                mscs_h.append(msc_h)
            # denominator chain: reciprocal on DVE straight out of PSUM,
            # then GpSimd partition-broadcast (split in halves so the first
            # fin muls start one broadcast earlier)
            rcp3 = sb_small.tile([1, NB], f32, tag="rcp3", bufs=1)
            nc.vector.reciprocal_approx_fast(rcp3[0:1, :], cs[0:1, :])
            bc3 = sb_small.tile([P, NB], f32, tag="bc3", bufs=1)
            for h in range(2):
                nc.gpsimd.partition_broadcast(
                    bc3[:, h * NBH:(h + 1) * NBH],
                    rcp3[0:1, h * NBH:(h + 1) * NBH], channels=P,
                )

            fins = [
                outfin_pool.tile([P, NB], f16, tag="fine", bufs=2,
                                 name=f"fine_{lt}")
                for lt in range(2)
            ]
            ops = {}
            for h in range(2):
                op = stp.tile([P, NB], f32, tag="stp", name=f"ope_{h}")
                for lt in range(2):
                    nc.tensor.matmul(
                        op[:, lt * NBH:(lt + 1) * NBH],
                        vu_bf[:, lt * P:(lt + 1) * P], mscs_h[h][:],
                        start=True, stop=True,
                    )
                ops[h] = op
            for h in range(2):
                for lt in range(2):
                    nc.vector.tensor_mul(
                        fins[lt][:, h * NBH:(h + 1) * NBH],
                        ops[h][:, lt * NBH:(lt + 1) * NBH],
                        bc3[:, h * NBH:(h + 1) * NBH],
                    )
                    if h == 1:
                        nc.sync.dma_start(
                            out_ext[lt * P:(lt + 1) * P, k3 * NB:(k3 + 1) * NB],
                            fins[lt][:],
                        )

    if not nc.is_finalized():
        nc.finalize()
    return nc


_GRAPH_CACHE = {}


def _get_graph():
    if "nc" not in _GRAPH_CACHE:
        _GRAPH_CACHE["nc"] = _build()
    return _GRAPH_CACHE["nc"]


def run(inputs: dict, trace: bool = False):
    """Run the SPMD kernel on 8 cores. Returns (output, BassKernelResults)."""
    from concourse.bass_utils import run_bass_kernel_spmd

    x = np.asarray(inputs["x"], dtype=np.float32)
    Q = np.asarray(inputs["Q"], dtype=np.float32)[0]
    K = np.asarray(inputs["K"], dtype=np.float32)[0]
    Vd = np.asarray(inputs["V_down"], dtype=np.float32)[0]
    Vu = np.asarray(inputs["V_up"], dtype=np.float32)[0]

    wqkv = np.ascontiguousarray(
        np.concatenate([Q, K, Vd], axis=1)
    ).astype(np.float16)
    vu = np.ascontiguousarray(Vu).astype(np.float16)

    in_maps = []
    for b in range(B):
        in_maps.append({
            "xT": np.ascontiguousarray(x[b].T).astype(np.float16),
            "Wqkv": wqkv,
            "Vu": vu,
        })

    nc = _get_graph()
    res = run_bass_kernel_spmd(nc, in_maps, core_ids=list(range(B)), trace=trace)
    # device output is [L, N] per core; un-transpose during the gather
    out = np.stack([np.asarray(res.results[i]["out"]).astype(np.float32).T for i in range(B)])
    return np.ascontiguousarray(out, dtype=np.float32), res


def kernel(**inputs) -> np.ndarray:
    out, _ = run(inputs, trace=False)
    return out
